# revision 12
# baseline (speedup 1.0000x reference)
"""Trainium2 Bass kernel: cached causal self-attention (dense transformer block).

Full module: y = CausalAttn(x; Wq, Wk, Wv) @ Wo.T + bo with
  B=4, S=2048, E=2048, H=16 heads, Dh=128, fp32 inputs.

Distribution: 8-way tensor parallel over heads (2 heads per NeuronCore).
Each core computes Q/K/V projections for its 2 heads, causal-softmax
attention, and a partial output projection; the host sums the 8 partials
and adds the bias.

All matmul operands are bf16 (PSUM accumulation stays fp32): same PE
streaming rate as float32r but Fast-Weight-Load halves the LDWEIGHTS
cost, SBUF/DMA traffic halves, and power throttling drops.  End-to-end
rel err ~6e-3, well inside the 2e-2 gate.

Layout: x pre-transposed on host (xT [E, B*S]); scores computed
transposed (sT[k, q]) so exp(sT) feeds attn@V directly with no on-chip
transpose.

Softmax denominators (this version): instead of a per-k-tile ones-vector
matmul (which costs the PE a third of the attention streaming), the exp
tiles are accumulated on the DVE into exsum[k, 2, q]; at chunk end ONE
all-ones [128,128] matmul per head partition-reduces exsum giving the
denominator already broadcast across all 128 partitions, so a direct
DVE reciprocal yields 1/den with no DRAM round-trip and the output
projection unblocks within ~2us of the last attention step.

Per k-step the PE runs s0,s1 (one 2-bank PSUM tile [128,2,512]),
av0,av1 and proj filler matmuls while ACT runs ONE merged exp of the
previous step's scores and the DVE accumulates exsum.  The output
projection accumulates BOTH heads into one PSUM bank (ctx is
pre-normalized by 1/den), and its matmul pairs are spread through the
attention steps and the next batch's QKV phase as PE filler work.
"""

import math

import ml_dtypes
import numpy as np

import concourse.bacc as bacc
import concourse.mybir as mybir
import concourse.tile as tile
from concourse.bass_utils import run_bass_kernel_spmd

F32 = mybir.dt.float32
BF16 = mybir.dt.bfloat16
F16 = mybir.dt.float16
AF = mybir.ActivationFunctionType
ALU = mybir.AluOpType

NEG = -1.0e30
# exp() output pre-scale 2^-6 (bias folded into the activation) keeps the
# fp16 exp-sum accumulators far from overflow; 1/den absorbs it exactly.
EXP_BIAS = -6.0 * math.log(2.0)

# Full-problem constants
EMB = 2048
N_HEADS = 16
HEAD_DIM = 128
B_FULL = 4
S_FULL = 2048
N_CORES = 8
HPC = N_HEADS // N_CORES  # heads per core = 2


def build(B=B_FULL, S=S_FULL, E=EMB, hpc=HPC, DH=HEAD_DIM, CH=512):
    """Build the per-core Bass program (same program on all 8 cores)."""
    assert hpc == 2
    SB = B * S
    DHC = hpc * DH          # per-core head dims (256)
    NE = E // 128           # e-tiles (contraction tiles)
    NEH = NE // 2
    NCH = S // CH           # 512-wide chunks per sequence
    KPC = CH // 128         # k-tiles per chunk (4)
    NST = S // 128          # 128-row s-tiles per sequence
    NOC = E // CH           # output chunks
    scale = 1.0 / math.sqrt(DH)

    nc = bacc.Bacc("TRN2", target_bir_lowering=False, debug=False,
                   num_devices=N_CORES)

    NCHB = (B * S) // CH
    xc = nc.dram_tensor("xc", [NCHB, 128, NE, CH], BF16, kind="ExternalInput")
    wqT = nc.dram_tensor("wqT", [128, NE, DHC], BF16, kind="ExternalInput")
    wkT = nc.dram_tensor("wkT", [128, NE, DHC], BF16, kind="ExternalInput")
    wvT = nc.dram_tensor("wvT", [128, NE, DHC], BF16, kind="ExternalInput")
    woT = nc.dram_tensor("woT", [128, hpc, E], BF16, kind="ExternalInput")
    masks = nc.dram_tensor("masks", [128, 2, 128], BF16, kind="ExternalInput")
    ones = nc.dram_tensor("ones", [128, 128], F16, kind="ExternalInput")
    ebias = nc.dram_tensor("ebias", [128, 1], F32, kind="ExternalInput")
    y = nc.dram_tensor("y", [SB, E], BF16, kind="ExternalOutput")

    with tile.TileContext(nc) as tc:
        with (
            tc.tile_pool(name="wpool", bufs=1) as wpool,
            tc.tile_pool(name="xtp", bufs=2) as xtp,
            tc.tile_pool(name="qpool", bufs=1) as qpool,
            tc.tile_pool(name="kvpool", bufs=2) as kvpool,
            tc.tile_pool(name="expp", bufs=4) as expp,
            tc.tile_pool(name="esp", bufs=2) as esp,
            tc.tile_pool(name="denp", bufs=2) as denp,
            tc.tile_pool(name="yp", bufs=4) as yp,
            tc.tile_pool(name="ps_sp", bufs=2, space="PSUM") as ps_sp,
            tc.tile_pool(name="ps_av", bufs=1, space="PSUM") as ps_av,
            tc.tile_pool(name="ps_pj", bufs=2, space="PSUM") as ps_pj,
        ):
            # Resident weights / constants
            wq_sb = wpool.tile([128, NE, DHC], BF16, tag="wq")
            wk_sb = wpool.tile([128, NE, DHC], BF16, tag="wk")
            wv_sb = wpool.tile([128, NE, DHC], BF16, tag="wv")
            wo_sb = wpool.tile([128, hpc, E], BF16, tag="wo")
            # interleave weight quarters with the first x chunk so the first
            # Q accumulation (wq + x) starts as early as possible
            x0a = xtp.tile([128, NEH, CH], BF16, tag="xta", name="x0a")
            x0b = xtp.tile([128, NEH, CH], BF16, tag="xtb", name="x0b")
            NEQ = NE // 4
            nc.sync.dma_start(wq_sb[:, 0:NEQ, :], wqT[:, 0:NEQ, :])
            nc.scalar.dma_start(x0a[:, 0:NEQ, :], xc[0, :, 0:NEQ, :])
            nc.gpsimd.dma_start(wq_sb[:, NEQ:NEH, :], wqT[:, NEQ:NEH, :])
            nc.sync.dma_start(x0a[:, NEQ:NEH, :], xc[0, :, NEQ:NEH, :])
            nc.sync.dma_start(wq_sb[:, NEH:NE, :], wqT[:, NEH:NE, :])
            nc.scalar.dma_start(x0b[:], xc[0, :, NEH:NE, :])
            nc.gpsimd.dma_start(wk_sb[:, 0:NEH, :], wkT[:, 0:NEH, :])
            nc.gpsimd.dma_start(wk_sb[:, NEH:NE, :], wkT[:, NEH:NE, :])
            xpre = ((0, 0), x0a, x0b)
            nc.sync.dma_start(wv_sb[:], wvT[:, :, :])
            nc.sync.dma_start(wo_sb[:], woT[:, :, :])
            mask_sb = wpool.tile([128, 2, 128], BF16, tag="mask")
            nc.sync.dma_start(mask_sb[:], masks[:, :, :])
            ones_sb = wpool.tile([128, 128], F16, tag="ones")
            nc.sync.dma_start(ones_sb[:], ones[:, :])
            ebias_sb = wpool.tile([128, 1], F32, tag="ebias")
            nc.sync.dma_start(ebias_sb[:], ebias[:, :])

            evict_parity = [0]

            def emit_proj_tile(pctxn, st, oc, ps0, phase="A"):
                """One output tile [128 q, CH]: both heads accumulated into one
                PSUM bank, plain-copy evict, y DMA.  During attention (phase B)
                the ACT engine is exp-bound, so evicts go to the DVE; in the
                QKV phase they alternate ACT/DVE."""
                p = ps_pj.tile([128, CH], F32, tag="pj")
                o0 = oc * CH
                nc.tensor.matmul(p[:], pctxn[:, 0, st * 128:(st + 1) * 128],
                                 wo_sb[:, 0, o0:o0 + CH], start=True, stop=False)
                nc.tensor.matmul(p[:], pctxn[:, 1, st * 128:(st + 1) * 128],
                                 wo_sb[:, 1, o0:o0 + CH], start=False, stop=True)
                ysb = yp.tile([128, CH], BF16, tag="ysb")
                if (phase == "A" and evict_parity[0] % 2 == 0) or (
                        phase == "B" and evict_parity[0] % 4 == 0):
                    nc.scalar.copy(ysb[:], p[:])
                else:
                    nc.vector.tensor_copy(ysb[:], p[:])
                evict_parity[0] += 1
                nc.gpsimd.dma_start(
                    y[ps0 + st * 128:ps0 + (st + 1) * 128, o0:o0 + CH], ysb[:])

            # pending proj work from the previous batch's last chunk:
            # list of (ctxn_tile, st, ps0) emitted as filler during phase A
            pending = []

            for b in range(B):
                s0 = b * S
                qT = qpool.tile([128, hpc, S], BF16, tag="qT")
                ctxTn = qpool.tile([128, hpc, S], BF16, tag="ctxn")
                kT = kvpool.tile([128, hpc, S], BF16, tag="kT")
                v_sb = kvpool.tile([128, NST, DHC], F16, tag="v")

                # ---------------- Phase A: Q/K/V projections -------------
                fillers = list(pending)
                pending = []
                fi = 0
                n_groups = NCH * (2 * hpc + KPC)
                gi = 0

                def maybe_fill_a():
                    nonlocal fi, gi
                    gi += 1
                    gd, nd = gi - 3, n_groups - 3
                    while fi < len(fillers) and gd >= 1 and fi + 1 <= (
                            len(fillers) * gd + nd - 1) // nd:
                        pctxn, st, ps0, oc = fillers[fi]
                        emit_proj_tile(pctxn, st, oc, ps0)
                        fi += 1

                for ch in range(NCH):
                    c0 = ch * CH
                    ci = b * NCH + ch
                    if xpre is not None and xpre[0] == (b, ch):
                        xta, xtb = xpre[1], xpre[2]
                    else:
                        xta = xtp.tile([128, NEH, CH], BF16, tag="xta")
                        nc.sync.dma_start(xta[:], xc[ci, :, 0:NEH, :])
                        xtb = xtp.tile([128, NEH, CH], BF16, tag="xtb")
                        nc.sync.dma_start(xtb[:], xc[ci, :, NEH:NE, :])
                    if ci + 1 < NCHB:
                        nb_, nch = (b, ch + 1) if ch + 1 < NCH else (b + 1, 0)
                        xna = xtp.tile([128, NEH, CH], BF16, tag="xta",
                                       name="xna")
                        nc.sync.dma_start(xna[:], xc[ci + 1, :, 0:NEH, :])
                        xnb = xtp.tile([128, NEH, CH], BF16, tag="xtb",
                                       name="xnb")
                        nc.sync.dma_start(xnb[:], xc[ci + 1, :, NEH:NE, :])
                        xpre = ((nb_, nch), xna, xnb)
                    else:
                        xpre = None

                    def xslice(et, lo=None, hi=None):
                        t = xta if et < NEH else xtb
                        e = et if et < NEH else et - NEH
                        if lo is None:
                            return t[:, e, :]
                        return t[:, e, lo:hi]

                    for h in range(hpc):
                        qp = ps_pj.tile([128, CH], F32, tag="pj")
                        for et in range(NE):
                            nc.tensor.matmul(
                                qp[:], wq_sb[:, et, h * DH:(h + 1) * DH],
                                xslice(et),
                                start=(et == 0), stop=(et == NE - 1))
                        nc.scalar.activation(qT[:, h, c0:c0 + CH], qp[:],
                                             AF.Identity, scale=scale)
                        maybe_fill_a()
                        kp = ps_pj.tile([128, CH], F32, tag="pj")
                        for et in range(NE):
                            nc.tensor.matmul(
                                kp[:], wk_sb[:, et, h * DH:(h + 1) * DH],
                                xslice(et),
                                start=(et == 0), stop=(et == NE - 1))
                        nc.scalar.activation(kT[:, h, c0:c0 + CH], kp[:],
                                             AF.Identity)
                        maybe_fill_a()
                    for st in range(KPC):
                        vp = ps_pj.tile([128, DHC], F32, tag="pj")
                        for et in range(NE):
                            nc.tensor.matmul(
                                vp[:], xslice(et, st * 128, (st + 1) * 128),
                                wv_sb[:, et, :],
                                start=(et == 0), stop=(et == NE - 1))
                        nc.scalar.activation(v_sb[:, ch * KPC + st, :], vp[:],
                                             AF.Identity)
                        maybe_fill_a()
                # any leftover fillers
                while fi < len(fillers):
                    pctxn, st, ps0, oc = fillers[fi]
                    emit_proj_tile(pctxn, st, oc, ps0)
                    fi += 1

                # ------- Phase B: attention, one-step software pipeline -----
                # Per step the PE runs s(kt) then av(kt-1); the merged exp(kt)
                # on ACT hides under s(kt+1) + av(kt) + fillers.  Proj filler
                # emission is rate-capped (2 tiles per 3 steps; the last batch
                # 4 per 3) -- overflow tiles spill into the next batch's QKV
                # phase where the PE has ACT/DVE headroom to spare.
                bfill = []      # proj tiles available for in-B emission
                emitted = [0]
                budget = [0]
                rnum, rden_ = (4, 3) if b == B - 1 else (1, 2)

                def maybe_fill_b():
                    budget[0] += rnum
                    while bfill and emitted[0] + 1 <= budget[0] // rden_:
                        st, oc = bfill.pop(0)
                        emit_proj_tile(ctxTn, st, oc, s0, phase="B")
                        emitted[0] += 1

                for g in range(NCH):
                    nk = KPC * (g + 1)
                    if g > 0:
                        for st in range((g - 1) * KPC, g * KPC):
                            for oc in range(NOC):
                                bfill.append((st, oc))
                    avp = ps_av.tile([128, hpc, CH], F32, tag="av")
                    exsum = esp.tile([128, hpc, CH], F16, tag="exsum")
                    lagq = []
                    for kt in range(nk):
                        j = kt - (nk - KPC)
                        off = 128 * j if j > 0 else 0
                        sp = ps_sp.tile([128, hpc, CH], F32, tag="sp")
                        for h in range(hpc):
                            nc.tensor.matmul(
                                sp[:, h, off:],
                                kT[:, h, kt * 128:(kt + 1) * 128],
                                qT[:, h, g * CH + off:(g + 1) * CH],
                                start=True, stop=True)
                        if j >= 0:
                            # mask col c: masked iff c < p (strict tri);
                            # only the first 128 cols of the suffix can hit
                            nc.vector.tensor_add(sp[:, :, off:off + 128],
                                                 sp[:, :, off:off + 128],
                                                 mask_sb[:, :, :])
                        ex = expp.tile([128, hpc, CH], F16, tag="ex")
                        nc.scalar.activation(ex[:, :, off:], sp[:, :, off:],
                                             AF.Exp, bias=ebias_sb[:])
                        # delayed by >=2 steps so chunk g-1's den/normalize
                        # can land before its proj tiles hit the PE queue
                        if kt >= 2 and (b == B - 1 or kt < nk - 1):
                            maybe_fill_b()
                        lagq.append((ex, off, kt))
                        if len(lagq) > 1:
                            pex, poff, pkt = lagq.pop(0)
                            for h in range(hpc):
                                nc.tensor.matmul(
                                    avp[:, h, poff:],
                                    v_sb[:, pkt, h * DH:(h + 1) * DH],
                                    pex[:, h, poff:],
                                    start=(pkt == 0), stop=(pkt == nk - 1),
                                    skip_group_check=True)
                            if pkt == 0:
                                nc.vector.tensor_copy(exsum[:], pex[:])
                            else:
                                nc.vector.tensor_add(exsum[:, :, poff:],
                                                     exsum[:, :, poff:],
                                                     pex[:, :, poff:])
                    # drain the lagged steps
                    for pex, poff, pkt in lagq:
                        for h in range(hpc):
                            nc.tensor.matmul(
                                avp[:, h, poff:],
                                v_sb[:, pkt, h * DH:(h + 1) * DH],
                                pex[:, h, poff:],
                                start=(pkt == 0), stop=(pkt == nk - 1),
                                skip_group_check=True)
                        if pkt == 0:
                            nc.vector.tensor_copy(exsum[:], pex[:])
                        else:
                            nc.vector.tensor_add(exsum[:, :, poff:],
                                                 exsum[:, :, poff:],
                                                 pex[:, :, poff:])
                    # ---- chunk end: den via one all-ones matmul per head,
                    # direct reciprocal, evict + normalize ctx ----
                    dnp = ps_sp.tile([128, hpc, CH], F32, tag="sp", name="dnp")
                    for h in range(hpc):
                        nc.tensor.matmul(dnp[:, h, :], ones_sb[:],
                                         exsum[:, h, :],
                                         start=True, stop=True)
                    rdenb = denp.tile([128, hpc, CH], F32, tag="rdenb")
                    nc.vector.reciprocal_approx_fast(rdenb[:], dnp[:])
                    # normalize straight out of the AV PSUM banks (frees them)
                    nc.vector.tensor_tensor(
                        ctxTn[:, :, g * CH:(g + 1) * CH], avp[:],
                        rdenb[:], op=ALU.mult)
                # unemitted + last chunk's proj become next-batch fillers
                for st, oc in bfill:
                    pending.append((ctxTn, st, s0, oc))
                for st in range((NCH - 1) * KPC, NCH * KPC):
                    for oc in range(NOC):
                        pending.append((ctxTn, st, s0, oc))
            # tail: final batch's last-chunk proj
            for pctxn, st, ps0, oc in pending:
                emit_proj_tile(pctxn, st, oc, ps0)
    nc.finalize()
    return nc


def host_consts(CH=512):
    p = np.arange(128)[:, None]
    c = np.arange(128)[None, :]
    masks = np.where(c < p, np.float32(NEG), np.float32(0.0))
    masks2 = np.broadcast_to(masks[:, None, :], (128, 2, 128))
    return {
        "masks": np.ascontiguousarray(masks2.astype(ml_dtypes.bfloat16)),
        "ones": np.ones((128, 128), dtype=np.float16),
        "ebias": np.full((128, 1), EXP_BIAS, dtype=np.float32),
    }


def host_inputs(x, Wq, Wk, Wv, Wo, B=B_FULL, S=S_FULL, E=EMB, hpc=HPC,
                DH=HEAD_DIM, CH=512):
    """Shard + lay out the full inputs for the 8 cores (bf16)."""
    SB = B * S
    DHC = hpc * DH
    NE = E // 128
    NCHB = SB // CH
    # chunk-major x so every per-chunk DMA is fully contiguous
    xT = x.reshape(SB, E).T.astype(ml_dtypes.bfloat16)
    xcb = np.ascontiguousarray(
        xT.reshape(NE, 128, NCHB, CH).transpose(2, 1, 0, 3))

    def wlay(w):  # [E, DHC] -> [128, NE, DHC], partition-major contiguous
        return np.ascontiguousarray(
            w.astype(ml_dtypes.bfloat16).reshape(NE, 128, -1).transpose(1, 0, 2))

    consts = host_consts(CH)

    in_maps = []
    for c in range(N_CORES):
        lo, hi = c * DHC, (c + 1) * DHC
        woc = Wo[:, lo:hi].T.astype(ml_dtypes.bfloat16)  # [DHC, E]
        in_maps.append({
            "xc": xcb,
            "wqT": wlay(Wq[lo:hi, :].T),
            "wkT": wlay(Wk[lo:hi, :].T),
            "wvT": wlay(Wv[lo:hi, :].T),
            "woT": np.ascontiguousarray(
                woc.reshape(hpc, 128, E).transpose(1, 0, 2)),
            **consts,
        })
    return in_maps


def kernel(x, Wq, Wk, Wv, Wo, bo):
    x = np.asarray(x, dtype=np.float32)
    Wq = np.asarray(Wq, dtype=np.float32)
    Wk = np.asarray(Wk, dtype=np.float32)
    Wv = np.asarray(Wv, dtype=np.float32)
    Wo = np.asarray(Wo, dtype=np.float32)
    bo = np.asarray(bo, dtype=np.float32)

    nc = build()
    in_maps = host_inputs(x, Wq, Wk, Wv, Wo)
    res = run_bass_kernel_spmd(nc, in_maps, list(range(N_CORES)))
    y = res.results[0]["y"].astype(np.float64)
    for c in range(1, N_CORES):
        y += res.results[c]["y"].astype(np.float64)
    y = (y + bo).astype(np.float32)
    return y.reshape(B_FULL, S_FULL, EMB)
